# revision 1
# baseline (speedup 1.0000x reference)
"""Multi-head causal attention (b=2, t=2048, d=1024, H=16, hd=64) on 8 TRN2 cores.

Sharding: tensor-parallel over heads — 2 heads per core. Each core:
  * projects x against its Wq/Wk/Wv column slice (128 dims = 2 heads)
  * runs causal attention for its 2 heads (scores kept transposed
    [k, q] so the ctx matmul contracts k on partitions; softmax row-sums
    come for free from a ones-column appended to V)
  * multiplies ctx by its Wo row slice -> a partial [4096, 1024] output
Host sums the 8 partials and adds the bias.

Matmuls run in float32r (full PE rate at N>=512, ~1e-4 relative error).
x is transposed on-chip with PE transpose-mode (fp32 exact).
Phases are interleaved per batch so batch-1 projections overlap batch-0
attention; PSUM eviction work is split across ACT and DVE.
"""

import sys
from contextlib import ExitStack

for _p in ("/opt/trn_rl_repo",):
    if _p not in sys.path:
        sys.path.insert(0, _p)

import numpy as np

import concourse.bass as bass
import concourse.tile as tile
from concourse import mybir
from concourse import bass_utils
from concourse.masks import make_identity

F32 = mybir.dt.float32
F32R = mybir.dt.float32r

P = 128          # partitions
B = 2            # batch
T = 2048         # seq len
NT = B * T       # 4096 tokens
DIN = 1024       # model dim
HD = 64          # head dim
NDC = DIN // P   # 8 d_in chunks
NSPB = T // 512  # 4 token stripes per batch (projection) / q-stripes
VN_W = 130       # vn cols per tok-tile: [h0 dims+ones, h1 dims+ones]
NTT = NT // P    # 32 token tiles

N_CORES = 8


def _split_multi_waits(nc, max_waits=1):
    """walrus in this container caps sync waits per instruction; spill
    extra waits onto same-engine NoOps inserted right before."""
    uid = 0
    for fn in nc.m.functions:
        for blk in fn.blocks:
            insts = blk.instructions
            new_list = []
            changed = False
            for inst in insts:
                si = inst.sync_info
                ow = list(si.on_wait) if si is not None and si.on_wait else []
                if len(ow) > max_waits:
                    spill, keep = ow[:-max_waits], ow[-max_waits:]
                    for w in spill:
                        nop = mybir.InstNoOp(name=f"I-wsplit-{blk.name}-{uid}", ins=[], outs=[])
                        uid += 1
                        nop.engine = inst.engine
                        nop.sync_info = mybir.SyncInfo(on_wait=[w], on_update=[])
                        new_list.append(nop)
                    inst.sync_info = mybir.SyncInfo(
                        on_wait=keep,
                        on_update=list(si.on_update) if si.on_update else [],
                    )
                    changed = True
                new_list.append(inst)
            if changed:
                insts[:] = new_list


def build(reps=1):
    nc = bass.Bass("TRN2", target_bir_lowering=False, debug=False, num_devices=N_CORES)
    xt = nc.dram_tensor("xt", [DIN, NT], F32R, kind="ExternalInput").ap()
    wq = nc.dram_tensor("wq", [DIN, P], F32R, kind="ExternalInput").ap()
    wk = nc.dram_tensor("wk", [DIN, P], F32R, kind="ExternalInput").ap()
    wv = nc.dram_tensor("wv", [DIN, P], F32R, kind="ExternalInput").ap()
    wo = nc.dram_tensor("wo", [P, DIN], F32R, kind="ExternalInput").ap()
    out = nc.dram_tensor("out", [NT, DIN], F32, kind="ExternalOutput").ap()

    with tile.TileContext(nc) as tc:
        with (
            tc.tile_pool(name="const", bufs=1) as const,
            tc.tile_pool(name="persist", bufs=1) as persist,
            tc.tile_pool(name="xt_p", bufs=12) as xt_p,
            tc.tile_pool(name="vstage_p", bufs=3) as vstage_p,
            tc.tile_pool(name="e_p", bufs=8) as e_p,
            tc.tile_pool(name="r_p", bufs=2) as r_p,
            tc.tile_pool(name="osb_p", bufs=6) as osb_p,
        ):
            # ---- constants / weights ----
            ident = const.tile([P, P], F32)
            make_identity(nc, ident)
            ones1f = const.tile([1, P], F32)
            nc.gpsimd.memset(ones1f[:], 1.0)
            ones1 = const.tile([1, P], F32R)
            nc.vector.tensor_copy(ones1[:], ones1f[:])

            wq_sb = const.tile([P, NDC * P], F32R)
            wk_sb = const.tile([P, NDC * P], F32R)
            wv_sb = const.tile([P, NDC * P], F32R)
            for w_dram, w_sb in ((wq, wq_sb), (wk, wk_sb), (wv, wv_sb)):
                nc.sync.dma_start(
                    w_sb[:].rearrange("p (c n) -> p c n", c=NDC),
                    w_dram.rearrange("(c p) n -> p c n", c=NDC),
                )
            wo_sb = const.tile([P, DIN], F32R)
            nc.sync.dma_start(wo_sb[:], wo)

            # ---- persistent activations ----
            qT_sb = persist.tile([P, NT], F32R)   # rows: h0 dims 0-63, h1 64-127
            kT_sb = persist.tile([P, NT], F32R)
            vn_sb = persist.tile([P, NTT * VN_W], F32R)
            ctxT_sb = persist.tile([P, NT], F32R)

            # ones columns of vn (locals 64 and 129 of each tok-tile)
            ones_stage = const.tile([P, 64], F32)
            nc.gpsimd.memset(ones_stage[:], 1.0)
            ones_view = vn_sb[:].rearrange("p (t w) -> p t w", w=VN_W)[:, :, 64::65]
            nc.vector.tensor_copy(ones_view, ones_stage[:].rearrange("p (t w) -> p t w", w=2))

            for rep in range(reps):
                stA = ExitStack()
                psA = stA.enter_context(
                    tc.tile_pool(name=f"psA{rep}", bufs=1, space="PSUM"))
                for b in range(B):
                    for sl in range(NSPB):
                        s = b * NSPB + sl
                        qps = psA.tile([P, 512], F32, name="qps", tag="qps")
                        kps = psA.tile([P, 512], F32, name="kps", tag="kps")
                        vps = psA.tile([P, 512], F32, name="vps", tag="vps")
                        for c in range(NDC):
                            xt_sb = xt_p.tile([P, 512], F32R, name="xt_sb")
                            nc.sync.dma_start(
                                xt_sb[:],
                                xt[c * P:(c + 1) * P, s * 512:(s + 1) * 512])
                            st = dict(start=(c == 0), stop=(c == NDC - 1))
                            nc.tensor.matmul(qps[:], wq_sb[:, c * P:(c + 1) * P], xt_sb[:], **st)
                            nc.tensor.matmul(kps[:], wk_sb[:, c * P:(c + 1) * P], xt_sb[:], **st)
                            nc.tensor.matmul(vps[:], wv_sb[:, c * P:(c + 1) * P], xt_sb[:], **st)

                        nc.scalar.copy(qT_sb[:, s * 512:(s + 1) * 512], qps[:])
                        nc.scalar.copy(kT_sb[:, s * 512:(s + 1) * 512], kps[:])
                        vstage = vstage_p.tile([P, 512], F32, name="vstage")
                        nc.vector.tensor_copy(vstage[:], vps[:])
                        vt_ps = psA.tile([P, 512], F32, name="vt_ps", tag="vt", bufs=2)
                        for j in range(4):
                            nc.tensor.transpose(
                                vt_ps[:, j * P:(j + 1) * P],
                                vstage[:, j * P:(j + 1) * P], ident[:],
                            )
                        vn_dst = (
                            vn_sb[:, 4 * s * VN_W:(4 * s + 4) * VN_W]
                            .rearrange("p (t w) -> p t w", w=VN_W)[:, :, :130]
                            .rearrange("p t (h d) -> p t h d", h=2)[:, :, :, :64]
                        )
                        vt_src = vt_ps[:].rearrange("p (t h d) -> p t h d", t=4, h=2)
                        nc.vector.tensor_copy(vn_dst, vt_src)
                stA.close()
                stB = ExitStack()
                psB = stB.enter_context(
                    tc.tile_pool(name=f"psB{rep}", bufs=1, space="PSUM"))
                for b in range(B):
                    for qs in range(NSPB):
                        qoff = b * T + qs * 512
                        ctx0 = psB.tile([65, 512], F32, name="ctx0", tag="ctx0", bufs=1)
                        ctx1 = psB.tile([65, 512], F32, name="ctx1", tag="ctx1", bufs=1)
                        nkb = 4 * qs + 4
                        for kb in range(nkb):
                            ktt = b * (T // P) + kb
                            m = kb - 4 * qs  # diag offset; >=0 on diag-crossing blocks
                            c0 = min(128 * m, 256) if m >= 0 else 0
                            spair = psB.tile([P, 1024], F32, name="spair", tag="spair", bufs=2)
                            for h in range(2):
                                nc.tensor.matmul(
                                    spair[:, h * 512 + c0:(h + 1) * 512],
                                    kT_sb[64 * h:64 * h + 64, ktt * P:(ktt + 1) * P],
                                    qT_sb[64 * h:64 * h + 64, qoff + c0:qoff + 512],
                                    start=True, stop=True,
                                )
                            e_sb = e_p.tile([P, 1024], F32R, name="e_sb")
                            if c0 == 0:
                                nc.scalar.activation(
                                    e_sb[:], spair[:], mybir.ActivationFunctionType.Exp,
                                    scale=0.125,
                                )
                            else:
                                for h in range(2):
                                    nc.scalar.activation(
                                        e_sb[:, h * 512 + c0:(h + 1) * 512],
                                        spair[:, h * 512 + c0:(h + 1) * 512],
                                        mybir.ActivationFunctionType.Exp, scale=0.125,
                                    )
                            if m >= 0:
                                # zero q < k inside the diag-crossing region
                                a0 = 128 * m          # first not-fully-masked column
                                lo = c0               # slice start used downstream
                                for h in range(2):
                                    sl_ = slice(h * 512 + lo, h * 512 + min(a0 + 128, 512))
                                    nc.gpsimd.affine_select(
                                        out=e_sb[:, sl_], in_=e_sb[:, sl_],
                                        compare_op=mybir.AluOpType.is_ge,
                                        fill=0.0, base=lo - 128 * m,
                                        pattern=[[1, sl_.stop - sl_.start]],
                                        channel_multiplier=-1,
                                    )
                            st = dict(start=(kb == 0), stop=(kb == nkb - 1))
                            nc.tensor.matmul(
                                ctx0[:, c0:512], vn_sb[:, ktt * VN_W:ktt * VN_W + 65],
                                e_sb[:, c0:512], **st,
                            )
                            nc.tensor.matmul(
                                ctx1[:, c0:512], vn_sb[:, ktt * VN_W + 65:(ktt + 1) * VN_W],
                                e_sb[:, 512 + c0:1024], **st,
                            )
                        # softmax denominators: row 64 of ctx psum = sum_k exp
                        rinv01 = r_p.tile([1, 1024], F32R, name="rinv01", tag="rinv01")
                        with nc.allow_low_precision(reason="f32r softmax denominators"):
                            nc.vector.reciprocal(rinv01[0:1, 0:512], ctx0[64:65, :])
                            nc.vector.reciprocal(rinv01[0:1, 512:1024], ctx1[64:65, :])
                        rb_ps = psB.tile([P, 1024], F32, name="rb_ps", tag="spair", bufs=2)
                        nc.tensor.matmul(rb_ps[:, 0:512], ones1[:], rinv01[0:1, 0:512],
                                         start=True, stop=True)
                        nc.tensor.matmul(rb_ps[:, 512:1024], ones1[:], rinv01[0:1, 512:1024],
                                         start=True, stop=True)
                        rb_sb = r_p.tile([P, 1024], F32, name="rb_sb", tag="rb_sb")
                        nc.vector.tensor_copy(rb_sb[:], rb_ps[:])
                        nc.vector.tensor_mul(
                            ctxT_sb[0:64, qoff:qoff + 512], ctx0[0:64, :], rb_sb[0:64, 0:512]
                        )
                        nc.vector.tensor_mul(
                            ctxT_sb[64:128, qoff:qoff + 512], ctx1[0:64, :], rb_sb[0:64, 512:1024]
                        )
                        # ---- partial out-projection for this q-stripe ----
                        for tcn in range(4 * qs, 4 * qs + 4):
                            coff = b * T + tcn * P
                            op01 = psB.tile([P, 1024], F32, name="op01", tag="op01", bufs=1)
                            nc.tensor.matmul(
                                op01[:, 0:512], ctxT_sb[:, coff:coff + P], wo_sb[:, 0:512],
                                start=True, stop=True,
                            )
                            nc.tensor.matmul(
                                op01[:, 512:1024], ctxT_sb[:, coff:coff + P], wo_sb[:, 512:1024],
                                start=True, stop=True,
                            )
                            osb = osb_p.tile([P, DIN], F32, name="osb")
                            nc.vector.tensor_copy(osb[:, 0:512], op01[:, 0:512])
                            nc.vector.tensor_copy(osb[:, 512:1024], op01[:, 512:1024])
                            nc.sync.dma_start(out[coff:coff + P, :], osb[:])

                stB.close()

    _split_multi_waits(nc)
    return nc


_NC_CACHE = None


def _get_nc():
    global _NC_CACHE
    if _NC_CACHE is None:
        _NC_CACHE = build()
    return _NC_CACHE


def make_in_maps(x, Wq, Wk, Wv, Wo):
    xt = np.ascontiguousarray(np.asarray(x, dtype=np.float32).reshape(NT, DIN).T)
    in_maps = []
    for c in range(N_CORES):
        sl = slice(c * P, (c + 1) * P)
        in_maps.append({
            "xt": xt,
            "wq": np.ascontiguousarray(np.asarray(Wq, dtype=np.float32)[:, sl]),
            "wk": np.ascontiguousarray(np.asarray(Wk, dtype=np.float32)[:, sl]),
            "wv": np.ascontiguousarray(np.asarray(Wv, dtype=np.float32)[:, sl]),
            "wo": np.ascontiguousarray(np.asarray(Wo, dtype=np.float32)[sl, :]),
        })
    return in_maps


def kernel(x, Wq, Wk, Wv, Wo, bo):
    nc = _get_nc()
    in_maps = make_in_maps(x, Wq, Wk, Wv, Wo)
    res = bass_utils.run_bass_kernel_spmd(
        nc, in_maps, core_ids=list(range(N_CORES)), trace=False
    )
    acc = np.zeros((NT, DIN), dtype=np.float64)
    for r in res.results:
        acc += r["out"].astype(np.float64)
    acc += np.asarray(bo, dtype=np.float64)[None, :]
    return acc.astype(np.float32).reshape(B, T, DIN)


if __name__ == "__main__":
    rng = np.random.default_rng(0)
    x = rng.standard_normal((B, T, DIN)).astype(np.float32)
    Wq = rng.standard_normal((DIN, DIN)).astype(np.float32) * 0.02
    Wk = rng.standard_normal((DIN, DIN)).astype(np.float32) * 0.02
    Wv = rng.standard_normal((DIN, DIN)).astype(np.float32) * 0.02
    Wo = rng.standard_normal((DIN, DIN)).astype(np.float32) * 0.02
    bo = np.zeros(DIN, dtype=np.float32)
    out = kernel(x=x, Wq=Wq, Wk=Wk, Wv=Wv, Wo=Wo, bo=bo)
    print("out", out.shape, out.dtype, float(np.abs(out).max()))

